# revision 33
# baseline (speedup 1.0000x reference)
"""Multi-head attention (B=4, S=2048, E=1024, H=16) on 8 Trainium2 cores.

Sharding: core c = (batch b = c//2, head-group g = c%2 of 8 heads).
Host-side prep per core:
  - q/k/v transposed to [E, S] (bf16) so every on-chip matmul contracts the
    partition dim with zero on-chip transposes,
  - k/v gathered to the unmasked key positions (attention is
    permutation-invariant over keys; masked keys contribute exactly 0),
    padded to KP (multiple of 128) with zero columns,
  - weight column/row slices for the 8-head group,
  - bv/bo folded into a single post-projection bias cb = bv_g @ wo_g (+ bo).
Each core computes a partial [S, E] output (its head-group's share of the
out-projection); the host sums the two partials per batch.

On-chip pipeline (all layouts transposed, S on the free dim):
  QT/KT = (x @ w + b)^T via PE (bf16) -> bf16, V likewise -> bf16,
  scores^T[k, q] per head (bf16, K=64), written to 2-bank PSUM tiles,
  exp = Exp(scores/8) straight from PSUM in [128, 1024] ACT ops -> bf16,
  attn@V with a ones column appended to V (M=65) so row 64 accumulates the
  softmax denominator for free (bf16).
  Zero-padded keys contribute exp(0)=1 to the denominator only; the exact
  pad count is subtracted from the sums (V pad rows are zero).
  Denominators: per q-block, the 8 head sum-rows are staged into one row,
  DMA'd into a [128, 32] tile (DVE reciprocal time scales with the free
  dim, so a [1, 4096] reciprocal is ~100x slower than [128, 32]),
  reciprocaled once, DMA'd back, and broadcast across partitions with K=1
  outer-product matmuls. One DVE multiply normalizes each head.
  final = aoT^T x wo (fp32r) + cb, DMA out in natural [S, E] layout.

Built on bacc.Bacc + nc.compile(): generate_event_semaphores() legalizes
the TRN2 one-sync-wait-per-instruction constraint.
"""

import sys

if "/opt/trn_rl_repo" not in sys.path:
    sys.path.insert(0, "/opt/trn_rl_repo")

import numpy as np
import ml_dtypes

import concourse.bass as bass
import concourse.tile as tile
from concourse import bacc, mybir
from concourse.bass_utils import run_bass_kernel_spmd

B, S, E, H = 4, 2048, 1024, 16
D = 64
EH = 512  # out-features per core (8 heads x 64)
H8 = 8  # heads per core
P = 128
QB = 512  # attention q-block (free dim of scores/exp tiles)
NQB = S // QB
SBLK = 512  # phase-B q-stream block
F32 = mybir.dt.float32
F32R = mybir.dt.float32r
BF16 = mybir.dt.bfloat16
AF = mybir.ActivationFunctionType
SCALE = 1.0 / 8.0  # 1/sqrt(D)

TRACE = False  # test.py flips this to get an NTFF profile
TMPDIR = None


def _blocks(total, sz):
    out = []
    off = 0
    while off < total:
        out.append((off, min(sz, total - off)))
        off += sz
    return out


def build(KP):
    KT = KP // P
    nc = bacc.Bacc("TRN2", target_bir_lowering=False, debug=False, num_devices=8)

    qTd = nc.dram_tensor("qT", [E, S], BF16, kind="ExternalInput").ap()
    kTd = nc.dram_tensor("kT", [E, KP], BF16, kind="ExternalInput").ap()
    vTd = nc.dram_tensor("vT", [E, KP], BF16, kind="ExternalInput").ap()
    wqd = nc.dram_tensor("wq", [E, EH], BF16, kind="ExternalInput").ap()
    wkd = nc.dram_tensor("wk", [E, EH], BF16, kind="ExternalInput").ap()
    wvd = nc.dram_tensor("wv", [E, EH], BF16, kind="ExternalInput").ap()
    wod = nc.dram_tensor("wo", [EH, E], BF16, kind="ExternalInput").ap()
    # one blob for all small fp32 constants -> ONE DMA -> ONE semaphore.
    # columns: [bq 4 | bk 4 | -n_pads 1 | cb E], replicated on all partitions
    cbd = nc.dram_tensor("cblob", [P, 9 + E], F32, kind="ExternalInput").ap()
    onesd = nc.dram_tensor("onesr", [1, 64], F32R, kind="ExternalInput").ap()
    outd = nc.dram_tensor("out", [S, E], F32, kind="ExternalOutput").ap()
    # DRAM bounce buffers for the sums/reciprocal partition reshuffle
    shopd = nc.dram_tensor("sums_hop", [NQB, H8 * QB], F32).ap()
    rhopd = nc.dram_tensor("rec_hop", [NQB, H8 * QB], F32R).ap()

    with tile.TileContext(nc) as tc:
        with (
            tc.tile_pool(name="consts", bufs=1) as consts,
            tc.tile_pool(name="persist", bufs=1) as persist,
            tc.tile_pool(name="work", bufs=1) as work,
            tc.tile_pool(name="pp", bufs=1, space="PSUM") as pp,
        ):
            ones64 = consts.tile([P, 64], F32R)
            # fp32r memset is invalid ISA; row 64 is the only row used
            nc.sync.dma_start(out=ones64[64:65, :], in_=onesd)
            cblob = consts.tile([P, 9 + E], F32)
            nc.sync.dma_start(out=cblob, in_=cbd)
            bq_sb = cblob[:, 0:4]
            bk_sb = cblob[:, 4:8]
            negnp = cblob[:, 8:9]
            cb_sb = cblob[:, 9:9 + E]

            QT = persist.tile([P, 4, S], BF16)
            KTt = persist.tile([P, 4, KP], BF16)
            V65 = persist.tile([P, KT, H8, 65], BF16)
            wo_sb = persist.tile([P, 4, E], BF16)
            nc.sync.dma_start(out=wo_sb, in_=wod.rearrange("(t p) e -> p t e", p=P))
            # col 64 of every head block must be 1.0 (softmax denominator
            # accumulator); strided memset is invalid ISA, so set the whole
            # tile and let the V copies overwrite cols 0..63.
            nc.vector.memset(V65, 1.0)

            # ---------------- phase B: projections ----------------
            with (
                tc.tile_pool(name="wpool", bufs=1) as wp,
                tc.tile_pool(name="stream", bufs=1) as strm,
            ):
                wq_sb = wp.tile([P, 8, EH], BF16, tag="w", bufs=1)
                nc.sync.dma_start(out=wq_sb, in_=wqd.rearrange("(t p) n -> p t n", p=P))
                for off, blk in _blocks(S, SBLK):
                    qsb = strm.tile([P, 8, SBLK], BF16, tag="xs", bufs=2)
                    hb = blk // 2
                    nc.sync.dma_start(
                        out=qsb[:, :, 0:hb],
                        in_=qTd[:, off:off + hb].rearrange("(t p) n -> p t n", p=P),
                    )
                    nc.sync.dma_start(
                        out=qsb[:, :, hb:blk],
                        in_=qTd[:, off + hb:off + blk].rearrange("(t p) n -> p t n", p=P),
                    )
                    for m in range(4):
                        ps = pp.tile([P, SBLK], F32, tag="scat", bufs=2)
                        for kk in range(8):
                            nc.tensor.matmul(
                                ps[:, 0:blk], wq_sb[:, kk, m * P:(m + 1) * P], qsb[:, kk, 0:blk],
                                start=(kk == 0), stop=(kk == 7),
                            )
                        nc.vector.tensor_scalar_add(
                            out=QT[:, m, off:off + blk], in0=ps[:, 0:blk],
                            scalar1=bq_sb[:, m:m + 1],
                        )

                wk_sb = wp.tile([P, 8, EH], BF16, tag="w", bufs=1)
                nc.sync.dma_start(out=wk_sb, in_=wkd.rearrange("(t p) n -> p t n", p=P))
                for off, blk in _blocks(KP, SBLK):
                    ksb = strm.tile([P, 8, SBLK], BF16, tag="xs", bufs=2)
                    hb = blk // 2
                    nc.sync.dma_start(
                        out=ksb[:, :, 0:hb],
                        in_=kTd[:, off:off + hb].rearrange("(t p) n -> p t n", p=P),
                    )
                    nc.sync.dma_start(
                        out=ksb[:, :, hb:blk],
                        in_=kTd[:, off + hb:off + blk].rearrange("(t p) n -> p t n", p=P),
                    )
                    for m in range(4):
                        ps = pp.tile([P, SBLK], F32, tag="scat", bufs=2)
                        for kk in range(8):
                            nc.tensor.matmul(
                                ps[:, 0:blk], wk_sb[:, kk, m * P:(m + 1) * P], ksb[:, kk, 0:blk],
                                start=(kk == 0), stop=(kk == 7),
                            )
                        nc.vector.tensor_scalar_add(
                            out=KTt[:, m, off:off + blk], in0=ps[:, 0:blk],
                            scalar1=bk_sb[:, m:m + 1],
                        )

                # V: V[k-row, h*64+d] (bias bv folded into cb on the host)
                wv_sb = wp.tile([P, 8, EH], BF16, tag="w", bufs=1)
                nc.sync.dma_start(out=wv_sb, in_=wvd.rearrange("(t p) n -> p t n", p=P))
                for vb in range(KT):
                    vsb = strm.tile([P, 8, P], BF16, tag="vs", bufs=2)
                    nc.sync.dma_start(
                        out=vsb, in_=vTd[:, vb * P:(vb + 1) * P].rearrange("(t p) n -> p t n", p=P),
                    )
                    ps = pp.tile([P, EH], F32, tag="scat", bufs=2)
                    for kk in range(8):
                        nc.tensor.matmul(
                            ps, vsb[:, kk, :], wv_sb[:, kk, :],
                            start=(kk == 0), stop=(kk == 7),
                        )
                    nc.vector.tensor_copy(
                        out=V65[:, vb, :, 0:64], in_=ps.rearrange("p (h d) -> p h d", h=H8),
                    )

            # -------- phase C/D: attention + normalization + out-projection --------
            # k-tile chunks per (head, q-block): pairs share a 2-bank psum tile
            # so one ACT exp op covers [128, 2*QB] straight from PSUM.
            kchunks = []
            kt0 = 0
            while kt0 < KT:
                kchunks.append((kt0, min(2, KT - kt0)))
                kt0 += 2

            nsum = H8 * QB

            def emit_scores(st, h):
                qb = st["qb"]
                pl = 64 * (h % 2)
                mh = h // 2
                ex = work.tile([P, KT, QB], BF16, tag="ex", bufs=4, name=f"ex_{qb}_{h}")
                st["exs"][h] = ex
                for kc0, kcn in kchunks:
                    psc = pp.tile([P, 2 * QB], F32, tag="scat", bufs=2)
                    for j in range(kcn):
                        kt = kc0 + j
                        nc.tensor.matmul(
                            psc[:, j * QB:(j + 1) * QB],
                            KTt[pl:pl + 64, mh, kt * P:(kt + 1) * P],
                            QT[pl:pl + 64, mh, qb * QB:(qb + 1) * QB],
                            start=True, stop=True,
                        )
                    nc.scalar.activation(
                        out=ex[:, kc0:kc0 + kcn, :], in_=psc[:, 0:kcn * QB],
                        func=AF.Exp, scale=SCALE,
                    )

            def emit_attnv(st, h):
                qb = st["qb"]
                ex = st["exs"].pop(h)
                po = pp.tile([P, QB], F32, tag="av", bufs=2)
                for kt in range(KT):
                    nc.tensor.matmul(
                        po[0:65, :], V65[:, kt, h, :], ex[:, kt, :],
                        start=(kt == 0), stop=(kt == KT - 1),
                    )
                # values out of PSUM right away (frees the bank); sums go to
                # DRAM per head (with the exact pad-count correction) for the
                # batched-reciprocal bounce.
                nc.vector.tensor_copy(out=st["pou"][:, h, :], in_=po[0:64, :])
                sr = work.tile([P, QB], F32, tag="srow", bufs=2)
                nc.vector.tensor_scalar_add(
                    out=sr[64:65, :], in0=po[64:65, :], scalar1=negnp[64:65, :],
                )
                nc.sync.dma_start(out=shopd[qb, h * QB:(h + 1) * QB], in_=sr[64:65, :])

            def emit_recip(st):
                # batched reciprocal: a [1, H8*QB] reciprocal is free-dim-bound
                # on DVE (~1.7us per 256 cols); bounce through DRAM to [128, 32]
                qb = st["qb"]
                sumT = work.tile([P, nsum // P], F32, tag="sumT", bufs=2)
                nc.sync.dma_start(
                    out=sumT, in_=shopd[qb, :].rearrange("(c p) -> p c", p=P),
                )
                recT = work.tile([P, nsum // P], F32R, tag="recT", bufs=2)
                with nc.allow_low_precision(reason="tf32 softmax denom is plenty"):
                    nc.vector.reciprocal(out=recT, in_=sumT)
                nc.sync.dma_start(
                    out=rhopd[qb, :].rearrange("(c p) -> p c", p=P), in_=recT,
                )

            def emit_norm_final(st):
                qb = st["qb"]
                aoT = st["aoT"]
                for h in range(H8):
                    po2 = h % 2
                    mh = h // 2
                    rr = work.tile([P, QB], F32R, tag="rr", bufs=2)
                    nc.sync.dma_start(
                        out=rr[64:65, :], in_=rhopd[qb, h * QB:(h + 1) * QB],
                    )
                    pb = pp.tile([64, QB], F32, tag="bc", bufs=1)
                    nc.tensor.matmul(
                        pb[0:64, :], ones64[64:65, :], rr[64:65, :],
                        start=True, stop=True,
                    )
                    pbs = work.tile([64, QB], F32, tag="pbs", bufs=2)
                    nc.vector.tensor_copy(pbs, pb[0:64, :])
                    if po2 == 0:
                        nc.vector.tensor_mul(out=aoT[0:64, mh, :], in0=st["pou"][:, h, :], in1=pbs)
                    else:
                        # DVE lanes are partition-locked; hop through a DMA to
                        # land the odd head at partitions 64..127.
                        ntmp = work.tile([64, QB], BF16, tag="ntmp", bufs=2)
                        nc.vector.tensor_mul(out=ntmp, in0=st["pou"][:, h, :], in1=pbs)
                        nc.sync.dma_start(out=aoT[64:128, mh, :], in_=ntmp)

                for mq in range(QB // P):
                    outsb = work.tile([P, E], F32, tag="osb", bufs=2)
                    for n2 in range(2):
                        pf = pp.tile([P, 512], F32, tag="fin", bufs=1)
                        for j in range(4):
                            nc.tensor.matmul(
                                pf,
                                aoT[:, j, mq * P:(mq + 1) * P],
                                wo_sb[:, j, n2 * 512:(n2 + 1) * 512],
                                start=(j == 0), stop=(j == 3),
                            )
                        nc.vector.tensor_add(
                            out=outsb[:, n2 * 512:(n2 + 1) * 512], in0=pf,
                            in1=cb_sb[:, n2 * 512:(n2 + 1) * 512],
                        )
                    r0 = qb * QB + mq * P
                    nc.sync.dma_start(out=outd[r0:r0 + P, :], in_=outsb)

            # Cross-block software pipeline. PE executes in emission order, so:
            # scores(h+1) lands before attnV(h) (which waits on ACT), and the
            # previous block's normalization + out-projection land between the
            # first heads of the next block (hiding the reciprocal DMA bounce).
            prev = None
            for qb in range(NQB):
                st = {
                    "qb": qb,
                    "exs": {},
                    "aoT": work.tile([P, 4, QB], BF16, tag="aoT", bufs=2, name=f"aoT_{qb}"),
                    "pou": work.tile([64, H8, QB], F32, tag="pou", bufs=2, name=f"pou_{qb}"),
                }
                # ~13us of PE work (4 scores + 2 attnV) before the previous
                # block's broadcast matmuls, which wait on the reciprocal's
                # 4-DMA bounce chain (~10us latency).
                emit_scores(st, 0)
                emit_scores(st, 1)
                emit_scores(st, 2)
                emit_scores(st, 3)
                emit_attnv(st, 0)
                emit_attnv(st, 1)
                if prev is not None:
                    emit_norm_final(prev)
                for h in range(4, H8):
                    emit_scores(st, h)
                    emit_attnv(st, h - 2)
                emit_attnv(st, H8 - 2)
                emit_attnv(st, H8 - 1)
                emit_recip(st)
                prev = st
            emit_norm_final(prev)

    nc.compile()
    return nc


_BUILD_CACHE = {}


def kernel(q, k, v, mask, wq, bq, wk, bk, wv, bv, wo, bo):
    q = np.asarray(q, np.float32)
    k = np.asarray(k, np.float32)
    v = np.asarray(v, np.float32)
    mask = np.asarray(mask)
    wq = np.asarray(wq, np.float32)
    bq = np.asarray(bq, np.float32)
    wk = np.asarray(wk, np.float32)
    bk = np.asarray(bk, np.float32)
    wv = np.asarray(wv, np.float32)
    bv = np.asarray(bv, np.float32)
    wo = np.asarray(wo, np.float32)
    bo = np.asarray(bo, np.float32)

    keep = mask.reshape(B, S) != 0
    idx = [np.nonzero(keep[b])[0] for b in range(B)]
    KP = max(256, max((len(ix) + P - 1) // P * P for ix in idx))

    if KP not in _BUILD_CACHE:
        _BUILD_CACHE[KP] = build(KP)
    nc = _BUILD_CACHE[KP]

    bf = ml_dtypes.bfloat16
    per_batch = []
    for b in range(B):
        ix = idx[b]
        qT = np.ascontiguousarray(q[b].T.astype(bf))
        kT = np.zeros((E, KP), bf)
        kT[:, : len(ix)] = k[b].T[:, ix].astype(bf)
        vT = np.zeros((E, KP), bf)
        vT[:, : len(ix)] = v[b].T[:, ix].astype(bf)
        per_batch.append((qT, kT, vT, KP - len(ix)))

    in_maps = []
    for c in range(8):
        b, g = divmod(c, 2)
        sl = slice(g * EH, (g + 1) * EH)
        qT, kT, vT, n_pads = per_batch[b]
        cb = bv[sl] @ wo[sl, :]
        if g == 0:
            cb = cb + bo
        cblob = np.zeros((P, 9 + E), np.float32)
        cblob[:, 0:4] = bq[sl].reshape(4, P).T
        cblob[:, 4:8] = bk[sl].reshape(4, P).T
        cblob[:, 8] = -float(n_pads)
        cblob[:, 9:] = cb.astype(np.float32)[None, :]
        in_maps.append(
            {
                "qT": qT,
                "kT": kT,
                "vT": vT,
                "wq": np.ascontiguousarray(wq[:, sl].astype(bf)),
                "wk": np.ascontiguousarray(wk[:, sl].astype(bf)),
                "wv": np.ascontiguousarray(wv[:, sl].astype(bf)),
                "wo": np.ascontiguousarray(wo[sl, :].astype(bf)),
                "cblob": cblob,
                "onesr": np.ones((1, 64), np.float32),
            }
        )

    res = run_bass_kernel_spmd(nc, in_maps, list(range(8)), trace=TRACE, tmpdir=TMPDIR)
    kernel.last_results = res
    outs = [r["out"] for r in res.results]
    out = np.stack([outs[2 * b] + outs[2 * b + 1] for b in range(B)])
    return out.astype(np.float32)
